# revision 52
# baseline (speedup 1.0000x reference)
"""NeuralFactorizationMachine Trainium2 kernel (8 NeuronCores, SPMD).

Reference computation (B=1024, N=16384, D=512, O=4096):
    sum_emb = sae @ emb                      (B, D)
    sum_sq  = (sae*sae) @ (emb*emb)          (B, D)
    inter   = 0.5*(sum_emb^2 - sum_sq)       (B, D)
    h       = relu(inter @ mlp1_w.T + b1)    (B, D)
    out     = h @ mlp2_w.T + b2 + sae @ linear_w.T + lb   (B, O)

Sharding (8 cores) — v2, collective-light:
  - FM is DATA-parallel over batch: each core computes sum_emb/sum_sq for
    its 128 batch rows over the FULL N using the full emb table. No
    AllReduce (the old N-sharded FM paid a ~225us 4MiB fp32 AllReduce).
  - linear / mlp2 are tensor-parallel over O (each core owns 512 output
    cols); final output is a host-side concat over O.
  - The only collective is a 1MiB bf16 AllGather of hT (the post-relu
    hidden layer), issued ~60us in and fully hidden under the linear GEMM.

Layouts: every heavy operand is PRE-TILED on the host into the exact
[128, ntiles, free] SBUF layout so all big DMAs are contiguous
(>=2KiB runs per partition, full 360GB/s, no descriptor penalty).

Phase order per core:
  1. FM: stream (saeb, emb) k-blocks; squares on DVE; 2x4 psum groups
     accumulate sum_embT/sum_sqT in [d, b] orientation (free dim = my 128
     batch cols) over 128 k-tiles.  interT = se^2 - sq (0.5 folded into
     mlp1 weights host-side), then mlp1 + relu+bias -> hT shard [D, 128].
  2. AllGather hT -> [8, D, 128] (every core now has h for all B).
  3. linear: stream (saeT, linwT) k-blocks, 8 psum groups accumulate
     out[m-tile, OS] over 128 k-tiles; then mlp2 (4 k-tiles of hT) and the
     rank-1 bias matmul accumulate into the SAME psum groups; one copy +
     store per m-tile.
All GEMM inputs are bf16 (fp32 PSUM accumulation).
"""

import numpy as np
import ml_dtypes

import concourse.bass as bass
import concourse.mybir as mybir
import concourse.tile as tile
from concourse import bacc
from concourse.bass_utils import run_bass_kernel_spmd

B, N, D, O = 1024, 16384, 512, 4096
C = 8                # cores
BS = B // C          # 128 batch rows per core (FM shard)
OS = O // C          # 512 output cols per core
BF16 = mybir.dt.bfloat16
F32 = mybir.dt.float32

NT = N // 128        # 128 k-tiles over the contraction dim
DT = D // 128        # 4 d-tiles
MT = B // 128        # 8 m-tiles (batch)
FMB = 16             # FM stream blocks
FILLK = 8            # linear blocks processed as FM-segment PE filler
F8 = mybir.dt.float8e4
FMT = NT // FMB      # 8 k-tiles per FM block
NB = 16              # linear stream blocks
NTB = NT // NB       # 8 k-tiles per linear block


def _build(phases=("fm", "ag", "lin", "mlp2")):
    nc = bacc.Bacc(
        "TRN2",
        target_bir_lowering=False,
        debug=False,
        enable_asserts=False,
        num_devices=C,
    )

    # fm_t packs [saeb | emb] along the free dim; lin_t packs [saeT | linwT].
    fm_t = nc.dram_tensor("fm_t", [128, NT, BS + D], BF16,
                          kind="ExternalInput").ap()
    lin_t = nc.dram_tensor("lin_t", [128, NT, B + OS], BF16,
                           kind="ExternalInput").ap()
    w1_t = nc.dram_tensor("w1_t", [128, DT, D], BF16, kind="ExternalInput").ap()
    b1_t = nc.dram_tensor("b1_t", [1, D], BF16, kind="ExternalInput").ap()
    w2_t = nc.dram_tensor("w2_t", [128, DT, OS], BF16, kind="ExternalInput").ap()
    biasrow = nc.dram_tensor("biasrow", [1, OS], BF16, kind="ExternalInput").ap()
    out = nc.dram_tensor("out", [B, OS], F32, kind="ExternalOutput").ap()

    with tile.TileContext(nc) as tc:
        with (
            tc.tile_pool(name="dram", bufs=1, space="DRAM") as dram,
            tc.tile_pool(name="wp", bufs=1) as wp,
            tc.tile_pool(name="p2s", bufs=3) as p2s,
        ):
            ag_in = dram.tile([D, BS], BF16, tag="ag_in", name="ag_in")
            ag_out = dram.tile([C, D, BS], BF16, tag="ag_out", name="ag_out",
                               addr_space="Shared")

            # small weights/constants on the scalar HWDGE queue so they never
            # queue behind the big sync-queue streams.
            w1 = wp.tile([128, DT, D], BF16, tag="w1", name="w1")
            nc.scalar.dma_start(w1[:], w1_t[:, :, :])
            b1 = wp.tile([1, D], BF16, tag="b1", name="b1")
            nc.scalar.dma_start(b1[:], b1_t[:, :])
            w2 = wp.tile([128, DT, OS], BF16, tag="w2", name="w2")
            with tc.tile_wait_until(0.15):
                nc.scalar.dma_start(w2[:], w2_t[:, :, :])
            br = wp.tile([1, OS], BF16, tag="br", name="br")
            nc.scalar.dma_start(br[:], biasrow[:, :])
            ones = wp.tile([1, 128], BF16, tag="ones", name="ones")
            nc.vector.memset(ones[:], 1.0)

            ht = wp.tile([128, DT, BS], BF16, tag="ht", name="ht")
            ht_all = wp.tile([128, DT, C, BS], BF16, tag="ht_all",
                             name="ht_all")

            # ---------------- Phase 1: FM (batch shard, full N) ------------
            # Linear block 0 is processed DURING the FM stream as PE filler:
            # m-sequential matmuls through 2 rotating PSUM banks, drained to
            # the SBUF accumulator acc.  FM's 8 accumulation groups are packed
            # into 2 PSUM banks so both coexist (4 of 8 banks used).
            lin_tiles = {}

            def load_lin_block(nb):
                ksl = slice(nb * NTB, (nb + 1) * NTB)
                t = p2s.tile([128, NTB, B + OS], BF16, tag="sl", name="sl")
                # Blocks >=1 are held back so the scheduler cannot hoist their
                # 9.5us transfers into the middle of the FM stream (which
                # would starve the FM PE wave); block 0 loads early as the
                # PE-filler operand.
                if nb == 0:
                    for q in range(4):   # finer deps: fills start sooner
                        nc.sync.dma_start(t[:, 2 * q:2 * q + 2, :],
                                          lin_t[:, 2 * q:2 * q + 2, :])
                elif nb < FILLK:
                    nc.sync.dma_start(t[:], lin_t[:, ksl, :])
                else:
                    with tc.tile_wait_until((125 + 9.0 * (nb - FILLK)) / 1000.0):
                        nc.sync.dma_start(t[:], lin_t[:, ksl, :])
                lin_tiles[nb] = t

            acc = wp.tile([128, MT, OS], F32, tag="acc", name="acc")

            # fill units are linear block-pairs: one psum accumulation and
            # one drain per (pair, m) — halves the drain traffic.
            FILL_UNITS = [(0, 1), (2, 3), (4, 5), (6, 7)]

            def filler(ju, m, fill_ps):
                blocks = FILL_UNITS[ju]
                ps = fill_ps.tile([128, OS], F32, tag="fill", name="fill")
                for h, nb in enumerate(blocks):
                    sl = lin_tiles[nb]
                    for nt in range(NTB):
                        nc.tensor.matmul(
                            ps[:],
                            sl[:, nt:nt + 1, m * 128:(m + 1) * 128],
                            sl[:, nt:nt + 1, B:B + OS],
                            start=(h == 0 and nt == 0),
                            stop=(h == len(blocks) - 1 and nt == NTB - 1),
                            skip_group_check=True,
                        )
                if ju == 0:
                    nc.scalar.copy(acc[:, m:m + 1, :], ps[:])
                else:
                    nc.vector.tensor_add(acc[:, m:m + 1, :],
                                         acc[:, m:m + 1, :], ps[:])

            if "fm" in phases:
                ident = wp.tile([128, 128], BF16, tag="ident", name="ident")
                from concourse.masks import make_identity
                make_identity(nc, ident[:])
                with (
                    tc.tile_pool(name="fms", bufs=3) as fms,
                    tc.tile_pool(name="fmps", bufs=2, space="PSUM") as fmps,
                    tc.tile_pool(name="fillps", bufs=2, space="PSUM") as fillps,
                    tc.tile_pool(name="fmst", bufs=4) as fmst,
                ):
                    # sum_emb / sum_sq in [my 128 batch rows, D] orientation:
                    # one full-bank psum accumulation group per GEMM.
                    pss = [fmps.tile([128, D], F32, tag="fmps",
                                     name=f"fmps{t}") for t in range(2)]
                    # small leading blocks so the PE wave starts sooner
                    sizes = [2, 2, 4] + [FMT] * (FMB - 1)
                    bounds = np.cumsum([0] + sizes)
                    nfill_units = 4 * MT if "lin" in phases else 0
                    fills = 0
                    next_lin = 0
                    nblk = len(sizes)
                    for blk, (k0, k1) in enumerate(zip(bounds[:-1],
                                                       bounds[1:])):
                        kn = k1 - k0
                        cb = fms.tile([128, FMT, BS + D], BF16, tag="cb",
                                      name="cb")
                        nc.sync.dma_start(cb[:, 0:kn, :], fm_t[:, k0:k1, :])
                        cq = fms.tile([128, FMT, BS + D], F8, tag="cq",
                                      name="cq")
                        # bf16 -> fp8 squares run at 1 elem/cycle (no DVE 2x
                        # mode with a 1-byte operand), so split across engines
                        nc.vector.tensor_mul(cq[:, 0:kn, 0:320],
                                             cb[:, 0:kn, 0:320],
                                             cb[:, 0:kn, 0:320])
                        nc.scalar.activation(
                            cq[:, 0:kn, 320:BS + D], cb[:, 0:kn, 320:BS + D],
                            mybir.ActivationFunctionType.Square)
                        if "lin" in phases and next_lin < FILLK and blk % 2 == 0:
                            load_lin_block(next_lin)
                            next_lin += 1
                        esl = slice(BS, BS + D)
                        for nt in range(kn):
                            nc.tensor.matmul(
                                pss[0][:],
                                cb[:, nt:nt + 1, 0:BS],
                                cb[:, nt:nt + 1, esl],
                                start=(k0 + nt == 0), stop=(k0 + nt == NT - 1),
                                skip_group_check=True,
                            )
                        # sum_sq in fp8 DoubleRow: 2 k-tiles per instruction
                        for nt in range(0, kn, 2):
                            nc.tensor.matmul(
                                pss[1][:],
                                cq[:, nt:nt + 2, 0:BS],
                                cq[:, nt:nt + 2, esl],
                                start=(k0 + nt == 0),
                                stop=(k0 + nt == NT - 2),
                                skip_group_check=True,
                                perf_mode=mybir.MatmulPerfMode.DoubleRow,
                            )
                        # spread the fill units across the stream (nb-major
                        # so each linear block's tile retires promptly)
                        ready_units = sum(
                            1 for u in FILL_UNITS if max(u) < next_lin)
                        target = (blk + 1) * nfill_units // nblk
                        while fills < min(target, ready_units * MT):
                            filler(fills // MT, fills % MT, fillps)
                            fills += 1
                    while fills < nfill_units:
                        filler(fills // MT, fills % MT, fillps)
                        fills += 1

                    # inter[b, d] = sum_emb^2 - sum_sq (0.5 folded into w1
                    # host-side), cast to bf16.  Square on the Act engine:
                    # a tensor op may read only ONE input from PSUM.
                    inter = wp.tile([128, D], BF16, tag="inter", name="inter")
                    tmp = fmst.tile([128, D], F32, tag="tmp", name="tmp")
                    nc.scalar.activation(tmp[:], pss[0][:],
                                         mybir.ActivationFunctionType.Square)
                    nc.vector.tensor_sub(inter[:], tmp[:], pss[1][:])

                # transpose inter -> interT tiles, then
                # hT[d2, b] = relu(mlp1wT.T @ interT + b1); bias applied via
                # rank-1 matmuls, d2 groups laid group-major in ONE psum bank
                # so a single relu activation covers all of hT.
                interT = wp.tile([128, DT, BS], BF16, tag="interT",
                                 name="interT")
                with tc.tile_pool(name="m1ps", bufs=2, space="PSUM") as m1ps:
                    for d in range(DT):
                        pt = m1ps.tile([128, BS], BF16, tag="trps",
                                       name="trps")
                        nc.tensor.transpose(
                            pt[:], inter[:, d * 128:(d + 1) * 128], ident[:])
                        nc.vector.tensor_copy(interT[:, d:d + 1, :], pt[:])
                    ps1 = m1ps.tile([128, DT * BS], F32, tag="m1ps",
                                    name="m1ps")
                    for d2 in range(DT):
                        osl = slice(d2 * BS, (d2 + 1) * BS)
                        for kd in range(DT):
                            nc.tensor.matmul(
                                ps1[:, osl],
                                w1[:, kd:kd + 1, d2 * 128:(d2 + 1) * 128],
                                interT[:, kd:kd + 1, :],
                                start=(kd == 0), stop=False,
                                skip_group_check=True,
                            )
                        nc.tensor.matmul(
                            ps1[:, osl], b1[0:1, d2 * 128:(d2 + 1) * 128], ones[:, :],
                            start=False, stop=True, skip_group_check=True,
                        )
                    nc.scalar.activation(
                        ht[:], ps1[:],
                        mybir.ActivationFunctionType.Relu,
                    )

                # ship my hT shard; AllGather; pull back the full hT.
                nc.gpsimd.dma_start(
                    ag_in.rearrange("(k p) b -> p k b", p=128), ht[:]
                )
                if "ag" in phases:
                    nc.gpsimd.collective_compute(
                        "AllGather",
                        mybir.AluOpType.bypass,
                        replica_groups=[list(range(C))],
                        ins=[ag_in.opt()],
                        outs=[ag_out.opt()],
                    )
                    for r in range(C):
                        nc.scalar.dma_start(
                            ht_all[:, :, r:r + 1, :],
                            ag_out[r:r + 1].rearrange(
                                "r (k p) b -> p (r k) b", p=128),
                        )
                else:
                    for r in range(C):
                        nc.scalar.dma_start(
                            ht_all[:, :, r:r + 1, :],
                            ag_in.rearrange("(k p) b -> p k b", p=128),
                        )

            # ---------------- Phase 2: linear GEMM + mlp2 (O shard) --------
            with (
                tc.tile_pool(name="p2ps", bufs=1, space="PSUM") as p2ps,
                tc.tile_pool(name="p2st", bufs=4) as p2st,
            ):
                psm = [p2ps.tile([128, OS], F32, tag=f"psm{m}", name=f"psm{m}")
                       for m in range(MT)]
                def mlp2_and_bias():
                    # accumulates into the open psm groups; hoisted mid-stream
                    # (order-independent) so it's off the critical-path tail.
                    for m in range(MT):
                        if "mlp2" in phases and "fm" in phases:
                            for kd in range(DT):
                                nc.tensor.matmul(
                                    psm[m][:],
                                    ht_all[:, kd:kd + 1, m:m + 1, :],
                                    w2[:, kd:kd + 1, :],
                                    start=False, stop=False,
                                    skip_group_check=True,
                                )
                        nc.tensor.matmul(
                            psm[m][:], ones[:, :], br[:, :],
                            start=False, stop=False, skip_group_check=True,
                        )

                if "lin" in phases:
                    nb0 = FILLK if "fm" in phases else 0
                    if "fm" in phases:
                        for j in range(FILLK):
                            lin_tiles.pop(j)  # consumed by the FM-phase filler
                    else:
                        nc.vector.memset(acc[:], 0.0)
                    for nb in range(nb0, NB):
                        if nb not in lin_tiles:
                            load_lin_block(nb)
                        sl = lin_tiles.pop(nb)
                        if nb < NB - 1:
                            for nt in range(NTB):
                                for m in range(MT):
                                    nc.tensor.matmul(
                                        psm[m][:],
                                        sl[:, nt:nt + 1,
                                           m * 128:(m + 1) * 128],
                                        sl[:, nt:nt + 1, B:B + OS],
                                        start=(nb == nb0 and nt == 0),
                                        stop=False,
                                        skip_group_check=True,
                                    )
                        else:
                            # last block m-major: each group's stop fires
                            # ~1.7us apart so the add+store pipeline drains
                            # under the remaining matmuls, not after them.
                            for m in range(MT):
                                for nt in range(NTB):
                                    nc.tensor.matmul(
                                        psm[m][:],
                                        sl[:, nt:nt + 1,
                                           m * 128:(m + 1) * 128],
                                        sl[:, nt:nt + 1, B:B + OS],
                                        start=False, stop=(nt == NTB - 1),
                                        skip_group_check=True,
                                    )
                                ot = p2st.tile([128, OS], F32, tag="ot",
                                               name="ot")
                                nc.vector.tensor_add(ot[:], psm[m][:],
                                                     acc[:, m:m + 1, :])
                                nc.sync.dma_start(
                                    out[m * 128:(m + 1) * 128, :], ot[:])
                        if nb == (nb0 + NB - 1) // 2:
                            mlp2_and_bias()
                else:
                    nc.vector.memset(acc[:], 0.0)
                    has_mlp2 = "mlp2" in phases and "fm" in phases
                    for m in range(MT):
                        nc.tensor.matmul(
                            psm[m][:], ones[:, :], br[:, :],
                            start=True, stop=not has_mlp2,
                            skip_group_check=True,
                        )
                        if has_mlp2:
                            for kd in range(DT):
                                nc.tensor.matmul(
                                    psm[m][:],
                                    ht_all[:, kd:kd + 1, m:m + 1, :],
                                    w2[:, kd:kd + 1, :],
                                    start=False, stop=(kd == DT - 1),
                                    skip_group_check=True,
                                )

                if "lin" not in phases:
                    for m in range(MT):
                        ot = p2st.tile([128, OS], F32, tag="ot", name="ot")
                        nc.vector.tensor_add(ot[:], psm[m][:],
                                             acc[:, m:m + 1, :])
                        nc.sync.dma_start(out[m * 128:(m + 1) * 128, :],
                                          ot[:])

    nc.compile()
    return nc


_CACHE = {}


def _get_nc():
    if "nc" not in _CACHE:
        _CACHE["nc"] = _build()
    return _CACHE["nc"]


def _tile128(a):
    """[N, F] -> [128, N//128, F] with [p, nt, f] = a[nt*128 + p, f]."""
    n, f = a.shape
    return np.ascontiguousarray(
        a.reshape(n // 128, 128, f).transpose(1, 0, 2))


def make_in_maps(sae_features, emb, linear_w, linear_b, mlp1_w, mlp1_b,
                 mlp2_w, mlp2_b):
    bf = ml_dtypes.bfloat16
    f32 = np.float32
    sae = np.asarray(sae_features, dtype=f32)
    emb = np.asarray(emb, f32)

    saeT = np.ascontiguousarray(sae.T).astype(bf)          # (N, B)
    emb_bf = emb.astype(bf)                                 # (N, D)
    # FM stream operands are pre-scaled by powers of 2 (4*sae, 128*emb) so
    # their on-device squares land in fp8 e4m3 range for the DoubleRow
    # sum_sq GEMM; the 2^18 product scale is folded into the mlp1 weights
    # (inter_scaled = (512*se)^2 - 2^18*sq = 2^18 * 2*inter).
    sae_fm = (4.0 * sae).astype(bf)
    emb_fm = (128.0 * emb).astype(bf)
    mlp1wT = np.ascontiguousarray(
        ((0.5 / 2.0 ** 18) * np.asarray(mlp1_w, f32)).T)
    w1_t = _tile128(mlp1wT.astype(bf))                      # [128, DT, D]
    b1_t = np.asarray(mlp1_b, f32).reshape(1, D).astype(bf)     # [1, D]
    mlp2wT_f = np.ascontiguousarray(np.asarray(mlp2_w, f32).T)   # (D, O)
    linwT_f = np.ascontiguousarray(np.asarray(linear_w, f32).T)  # (N, O)
    bias_f = np.asarray(linear_b, f32) + np.asarray(mlp2_b, f32)  # (O,)

    in_maps = []
    for c in range(C):
        osl = slice(c * OS, (c + 1) * OS)
        bsl = slice(c * BS, (c + 1) * BS)
        fm = np.concatenate([np.ascontiguousarray(sae_fm.T)[:, bsl],
                             emb_fm], axis=1)              # (N, BS+D)
        lin = np.concatenate(
            [saeT, linwT_f[:, osl].astype(bf)], axis=1)       # (N, B+OS)
        in_maps.append({
            "fm_t": _tile128(fm),
            "lin_t": _tile128(lin),
            "w1_t": w1_t,
            "b1_t": b1_t,
            "w2_t": _tile128(
                np.ascontiguousarray(mlp2wT_f[:, osl]).astype(bf)),
            "biasrow": bias_f[osl].reshape(1, OS).astype(bf),
        })
    return in_maps


def kernel(sae_features, emb, linear_w, linear_b, mlp1_w, mlp1_b, mlp2_w,
           mlp2_b):
    nc = _get_nc()
    in_maps = make_in_maps(
        sae_features, emb, linear_w, linear_b, mlp1_w, mlp1_b, mlp2_w, mlp2_b
    )
    res = run_bass_kernel_spmd(nc, in_maps, list(range(C)))
    full = np.empty((B, O), dtype=np.float32)
    for c in range(C):
        full[:, c * OS:(c + 1) * OS] = res.results[c]["out"]
    return full


# revision 55
# speedup vs baseline: 1.1067x; 1.1067x over previous
"""NeuralFactorizationMachine Trainium2 kernel (8 NeuronCores, SPMD).

Reference computation (B=1024, N=16384, D=512, O=4096):
    sum_emb = sae @ emb                      (B, D)
    sum_sq  = (sae*sae) @ (emb*emb)          (B, D)
    inter   = 0.5*(sum_emb^2 - sum_sq)       (B, D)
    h       = relu(inter @ mlp1_w.T + b1)    (B, D)
    out     = h @ mlp2_w.T + b2 + sae @ linear_w.T + lb   (B, O)

Sharding (8 cores) — v2, collective-light:
  - FM is DATA-parallel over batch: each core computes sum_emb/sum_sq for
    its 128 batch rows over the FULL N using the full emb table. No
    AllReduce (the old N-sharded FM paid a ~225us 4MiB fp32 AllReduce).
  - linear / mlp2 are tensor-parallel over O (each core owns 512 output
    cols); final output is a host-side concat over O.
  - The only collective is a 1MiB bf16 AllGather of hT (the post-relu
    hidden layer), issued ~60us in and fully hidden under the linear GEMM.

Layouts: every heavy operand is PRE-TILED on the host into the exact
[128, ntiles, free] SBUF layout so all big DMAs are contiguous
(>=2KiB runs per partition, full 360GB/s, no descriptor penalty).

Phase order per core:
  1. FM: stream (saeb, emb) k-blocks; squares on DVE; 2x4 psum groups
     accumulate sum_embT/sum_sqT in [d, b] orientation (free dim = my 128
     batch cols) over 128 k-tiles.  interT = se^2 - sq (0.5 folded into
     mlp1 weights host-side), then mlp1 + relu+bias -> hT shard [D, 128].
  2. AllGather hT -> [8, D, 128] (every core now has h for all B).
  3. linear: stream (saeT, linwT) k-blocks, 8 psum groups accumulate
     out[m-tile, OS] over 128 k-tiles; then mlp2 (4 k-tiles of hT) and the
     rank-1 bias matmul accumulate into the SAME psum groups; one copy +
     store per m-tile.
All GEMM inputs are bf16 (fp32 PSUM accumulation).
"""

import numpy as np
import ml_dtypes

import concourse.bass as bass
import concourse.mybir as mybir
import concourse.tile as tile
from concourse import bacc
from concourse.bass_utils import run_bass_kernel_spmd

B, N, D, O = 1024, 16384, 512, 4096
C = 8                # cores
BS = B // C          # 128 batch rows per core (FM shard)
OS = O // C          # 512 output cols per core
BF16 = mybir.dt.bfloat16
F32 = mybir.dt.float32

NT = N // 128        # 128 k-tiles over the contraction dim
DT = D // 128        # 4 d-tiles
MT = B // 128        # 8 m-tiles (batch)
FMB = 16             # FM stream blocks
FILLK = 8            # linear blocks processed as FM-segment PE filler
F8 = mybir.dt.float8e4
FMT = NT // FMB      # 8 k-tiles per FM block
NB = 16              # linear stream blocks
NTB = NT // NB       # 8 k-tiles per linear block


def _build(phases=("fm", "ag", "lin", "mlp2")):
    nc = bacc.Bacc(
        "TRN2",
        target_bir_lowering=False,
        debug=False,
        enable_asserts=False,
        num_devices=C,
    )

    # fm_t packs [saeb | emb] along the free dim; lin_t packs [saeT | linwT].
    fm_t = nc.dram_tensor("fm_t", [128, NT, BS + D], BF16,
                          kind="ExternalInput").ap()
    lin_t = nc.dram_tensor("lin_t", [128, NT, B + OS], BF16,
                           kind="ExternalInput").ap()
    w1_t = nc.dram_tensor("w1_t", [128, DT, D], BF16, kind="ExternalInput").ap()
    b1_t = nc.dram_tensor("b1_t", [1, D], BF16, kind="ExternalInput").ap()
    w2_t = nc.dram_tensor("w2_t", [128, DT, OS], BF16, kind="ExternalInput").ap()
    biasrow = nc.dram_tensor("biasrow", [1, OS], BF16, kind="ExternalInput").ap()
    out = nc.dram_tensor("out", [B, OS], F32, kind="ExternalOutput").ap()

    with tile.TileContext(nc) as tc:
        with (
            tc.tile_pool(name="dram", bufs=1, space="DRAM") as dram,
            tc.tile_pool(name="wp", bufs=1) as wp,
            tc.tile_pool(name="p2s", bufs=3) as p2s,
        ):
            ag_in = dram.tile([D, BS], BF16, tag="ag_in", name="ag_in")
            ag_out = dram.tile([C, D, BS], BF16, tag="ag_out", name="ag_out",
                               addr_space="Shared")

            # small weights/constants on the scalar HWDGE queue so they never
            # queue behind the big sync-queue streams.
            w1 = wp.tile([128, DT, D], BF16, tag="w1", name="w1")
            with tc.tile_wait_until(0.09):
                nc.scalar.dma_start(w1[:], w1_t[:, :, :])
            b1 = wp.tile([1, D], BF16, tag="b1", name="b1")
            nc.scalar.dma_start(b1[:], b1_t[:, :])
            w2 = wp.tile([128, DT, OS], BF16, tag="w2", name="w2")
            with tc.tile_wait_until(0.15):
                nc.scalar.dma_start(w2[:], w2_t[:, :, :])
            br = wp.tile([1, OS], BF16, tag="br", name="br")
            nc.scalar.dma_start(br[:], biasrow[:, :])
            ones = wp.tile([1, 128], BF16, tag="ones", name="ones")
            nc.vector.memset(ones[:], 1.0)

            ht = wp.tile([128, DT, BS], BF16, tag="ht", name="ht")
            ht_all = wp.tile([128, DT, C, BS], BF16, tag="ht_all",
                             name="ht_all")

            # ---------------- Phase 1: FM (batch shard, full N) ------------
            # Linear block 0 is processed DURING the FM stream as PE filler:
            # m-sequential matmuls through 2 rotating PSUM banks, drained to
            # the SBUF accumulator acc.  FM's 8 accumulation groups are packed
            # into 2 PSUM banks so both coexist (4 of 8 banks used).
            lin_tiles = {}

            def load_lin_block(nb):
                ksl = slice(nb * NTB, (nb + 1) * NTB)
                t = p2s.tile([128, NTB, B + OS], BF16, tag="sl", name="sl")
                # Blocks >=1 are held back so the scheduler cannot hoist their
                # 9.5us transfers into the middle of the FM stream (which
                # would starve the FM PE wave); block 0 loads early as the
                # PE-filler operand.
                if nb == 0:
                    for q in range(4):   # finer deps: fills start sooner
                        nc.sync.dma_start(t[:, 2 * q:2 * q + 2, :],
                                          lin_t[:, 2 * q:2 * q + 2, :])
                elif nb < FILLK:
                    nc.sync.dma_start(t[:], lin_t[:, ksl, :])
                else:
                    with tc.tile_wait_until((125 + 9.0 * (nb - FILLK)) / 1000.0):
                        nc.sync.dma_start(t[:], lin_t[:, ksl, :])
                lin_tiles[nb] = t

            acc = wp.tile([128, MT, OS], F32, tag="acc", name="acc")

            # fill units: single blocks early (they become ready as each
            # block lands), pairs later (halves the drain traffic).
            FILL_UNITS = [(0, 1), (2, 3), (4, 5), (6, 7)]

            def filler(ju, m, fill_ps):
                blocks = FILL_UNITS[ju]
                ps = fill_ps.tile([128, OS], F32, tag="fill", name="fill")
                for h, nb in enumerate(blocks):
                    sl = lin_tiles[nb]
                    for nt in range(NTB):
                        nc.tensor.matmul(
                            ps[:],
                            sl[:, nt:nt + 1, m * 128:(m + 1) * 128],
                            sl[:, nt:nt + 1, B:B + OS],
                            start=(h == 0 and nt == 0),
                            stop=(h == len(blocks) - 1 and nt == NTB - 1),
                            skip_group_check=True,
                        )
                if ju == 0:
                    nc.scalar.copy(acc[:, m:m + 1, :], ps[:])
                else:
                    nc.vector.tensor_add(acc[:, m:m + 1, :],
                                         acc[:, m:m + 1, :], ps[:])

            if "fm" in phases:
                ident = wp.tile([128, 128], BF16, tag="ident", name="ident")
                from concourse.masks import make_identity
                make_identity(nc, ident[:])
                with (
                    tc.tile_pool(name="fms", bufs=3) as fms,
                    tc.tile_pool(name="fmps", bufs=2, space="PSUM") as fmps,
                    tc.tile_pool(name="fillps", bufs=2, space="PSUM") as fillps,
                    tc.tile_pool(name="fmst", bufs=4) as fmst,
                ):
                    # sum_emb / sum_sq in [my 128 batch rows, D] orientation:
                    # one full-bank psum accumulation group per GEMM.
                    pss = [fmps.tile([128, D], F32, tag="fmps",
                                     name=f"fmps{t}") for t in range(2)]
                    # small leading blocks so the PE wave starts sooner
                    sizes = [2, 2, 4] + [FMT] * (FMB - 1)
                    bounds = np.cumsum([0] + sizes)
                    nfill_units = len(FILL_UNITS) * MT \
                        if "lin" in phases else 0
                    fills = 0
                    next_lin = 0
                    nblk = len(sizes)
                    for blk, (k0, k1) in enumerate(zip(bounds[:-1],
                                                       bounds[1:])):
                        kn = k1 - k0
                        cb = fms.tile([128, FMT, BS + D], BF16, tag="cb",
                                      name="cb")
                        nc.sync.dma_start(cb[:, 0:kn, :], fm_t[:, k0:k1, :])
                        cq = fms.tile([128, FMT, BS + D], F8, tag="cq",
                                      name="cq")
                        # bf16 -> fp8 squares run at 1 elem/cycle (no DVE 2x
                        # mode with a 1-byte operand), so split across engines
                        nc.vector.tensor_mul(cq[:, 0:kn, 0:320],
                                             cb[:, 0:kn, 0:320],
                                             cb[:, 0:kn, 0:320])
                        nc.scalar.activation(
                            cq[:, 0:kn, 320:BS + D], cb[:, 0:kn, 320:BS + D],
                            mybir.ActivationFunctionType.Square)
                        if "lin" in phases and next_lin < FILLK:
                            load_lin_block(next_lin)
                            next_lin += 1
                        esl = slice(BS, BS + D)
                        for nt in range(kn):
                            nc.tensor.matmul(
                                pss[0][:],
                                cb[:, nt:nt + 1, 0:BS],
                                cb[:, nt:nt + 1, esl],
                                start=(k0 + nt == 0), stop=(k0 + nt == NT - 1),
                                skip_group_check=True,
                            )
                        # sum_sq in fp8 DoubleRow: 2 k-tiles per instruction
                        for nt in range(0, kn, 2):
                            nc.tensor.matmul(
                                pss[1][:],
                                cq[:, nt:nt + 2, 0:BS],
                                cq[:, nt:nt + 2, esl],
                                start=(k0 + nt == 0),
                                stop=(k0 + nt == NT - 2),
                                skip_group_check=True,
                                perf_mode=mybir.MatmulPerfMode.DoubleRow,
                            )
                        # spread the fill units across the stream (nb-major
                        # so each linear block's tile retires promptly)
                        ready_units = sum(
                            1 for u in FILL_UNITS if max(u) < next_lin)
                        target = (blk + 1) * nfill_units // nblk
                        while fills < min(target, ready_units * MT):
                            filler(fills // MT, fills % MT, fillps)
                            fills += 1
                    while fills < nfill_units:
                        filler(fills // MT, fills % MT, fillps)
                        fills += 1

                    # inter[b, d] = sum_emb^2 - sum_sq (0.5 folded into w1
                    # host-side), cast to bf16.  Square on the Act engine:
                    # a tensor op may read only ONE input from PSUM.
                    inter = wp.tile([128, D], BF16, tag="inter", name="inter")
                    tmp = fmst.tile([128, D], F32, tag="tmp", name="tmp")
                    nc.scalar.activation(tmp[:], pss[0][:],
                                         mybir.ActivationFunctionType.Square)
                    nc.vector.tensor_sub(inter[:], tmp[:], pss[1][:])

                # transpose inter -> interT tiles, then
                # hT[d2, b] = relu(mlp1wT.T @ interT + b1); bias applied via
                # rank-1 matmuls, d2 groups laid group-major in ONE psum bank
                # so a single relu activation covers all of hT.
                interT = wp.tile([128, DT, BS], BF16, tag="interT",
                                 name="interT")
                with tc.tile_pool(name="m1ps", bufs=2, space="PSUM") as m1ps:
                    for d in range(DT):
                        pt = m1ps.tile([128, BS], BF16, tag="trps",
                                       name="trps")
                        nc.tensor.transpose(
                            pt[:], inter[:, d * 128:(d + 1) * 128], ident[:])
                        nc.vector.tensor_copy(interT[:, d:d + 1, :], pt[:])
                    ps1 = m1ps.tile([128, DT * BS], F32, tag="m1ps",
                                    name="m1ps")
                    for d2 in range(DT):
                        osl = slice(d2 * BS, (d2 + 1) * BS)
                        for kd in range(DT):
                            nc.tensor.matmul(
                                ps1[:, osl],
                                w1[:, kd:kd + 1, d2 * 128:(d2 + 1) * 128],
                                interT[:, kd:kd + 1, :],
                                start=(kd == 0), stop=False,
                                skip_group_check=True,
                            )
                        nc.tensor.matmul(
                            ps1[:, osl], b1[0:1, d2 * 128:(d2 + 1) * 128], ones[:, :],
                            start=False, stop=True, skip_group_check=True,
                        )
                    nc.scalar.activation(
                        ht[:], ps1[:],
                        mybir.ActivationFunctionType.Relu,
                    )

                # ship my hT shard; AllGather; pull back the full hT.
                nc.gpsimd.dma_start(
                    ag_in.rearrange("(k p) b -> p k b", p=128), ht[:]
                )
                if "ag" in phases:
                    nc.gpsimd.collective_compute(
                        "AllGather",
                        mybir.AluOpType.bypass,
                        replica_groups=[list(range(C))],
                        ins=[ag_in.opt()],
                        outs=[ag_out.opt()],
                    )
                    for r in range(C):
                        nc.scalar.dma_start(
                            ht_all[:, :, r:r + 1, :],
                            ag_out[r:r + 1].rearrange(
                                "r (k p) b -> p (r k) b", p=128),
                        )
                else:
                    for r in range(C):
                        nc.scalar.dma_start(
                            ht_all[:, :, r:r + 1, :],
                            ag_in.rearrange("(k p) b -> p k b", p=128),
                        )

            # ---------------- Phase 2: linear GEMM + mlp2 (O shard) --------
            with (
                tc.tile_pool(name="p2ps", bufs=1, space="PSUM") as p2ps,
                tc.tile_pool(name="p2st", bufs=4) as p2st,
            ):
                psm = [p2ps.tile([128, OS], F32, tag=f"psm{m}", name=f"psm{m}")
                       for m in range(MT)]
                def mlp2_and_bias():
                    # accumulates into the open psm groups; hoisted mid-stream
                    # (order-independent) so it's off the critical-path tail.
                    for m in range(MT):
                        if "mlp2" in phases and "fm" in phases:
                            for kd in range(DT):
                                nc.tensor.matmul(
                                    psm[m][:],
                                    ht_all[:, kd:kd + 1, m:m + 1, :],
                                    w2[:, kd:kd + 1, :],
                                    start=False, stop=False,
                                    skip_group_check=True,
                                )
                        nc.tensor.matmul(
                            psm[m][:], ones[:, :], br[:, :],
                            start=False, stop=False, skip_group_check=True,
                        )

                if "lin" in phases:
                    nb0 = FILLK if "fm" in phases else 0
                    if "fm" in phases:
                        for j in range(FILLK):
                            lin_tiles.pop(j)  # consumed by the FM-phase filler
                    else:
                        nc.vector.memset(acc[:], 0.0)
                    for nb in range(nb0, NB):
                        if nb not in lin_tiles:
                            load_lin_block(nb)
                        sl = lin_tiles.pop(nb)
                        if nb < NB - 1:
                            for nt in range(NTB):
                                for m in range(MT):
                                    nc.tensor.matmul(
                                        psm[m][:],
                                        sl[:, nt:nt + 1,
                                           m * 128:(m + 1) * 128],
                                        sl[:, nt:nt + 1, B:B + OS],
                                        start=(nb == nb0 and nt == 0),
                                        stop=False,
                                        skip_group_check=True,
                                    )
                        else:
                            # last block m-major: each group's stop fires
                            # ~1.7us apart so the add+store pipeline drains
                            # under the remaining matmuls, not after them.
                            for m in range(MT):
                                for nt in range(NTB):
                                    nc.tensor.matmul(
                                        psm[m][:],
                                        sl[:, nt:nt + 1,
                                           m * 128:(m + 1) * 128],
                                        sl[:, nt:nt + 1, B:B + OS],
                                        start=False, stop=(nt == NTB - 1),
                                        skip_group_check=True,
                                    )
                                ot = p2st.tile([128, OS], F32, tag="ot",
                                               name="ot")
                                for hh in range(2):
                                    osl2 = slice(hh * OS // 2,
                                                 (hh + 1) * OS // 2)
                                    nc.vector.tensor_add(
                                        ot[:, osl2], psm[m][:, osl2],
                                        acc[:, m:m + 1, osl2])
                                    nc.sync.dma_start(
                                        out[m * 128:(m + 1) * 128, osl2],
                                        ot[:, osl2])
                        if nb == (nb0 + NB - 1) // 2:
                            mlp2_and_bias()
                else:
                    nc.vector.memset(acc[:], 0.0)
                    has_mlp2 = "mlp2" in phases and "fm" in phases
                    for m in range(MT):
                        nc.tensor.matmul(
                            psm[m][:], ones[:, :], br[:, :],
                            start=True, stop=not has_mlp2,
                            skip_group_check=True,
                        )
                        if has_mlp2:
                            for kd in range(DT):
                                nc.tensor.matmul(
                                    psm[m][:],
                                    ht_all[:, kd:kd + 1, m:m + 1, :],
                                    w2[:, kd:kd + 1, :],
                                    start=False, stop=(kd == DT - 1),
                                    skip_group_check=True,
                                )

                if "lin" not in phases:
                    for m in range(MT):
                        ot = p2st.tile([128, OS], F32, tag="ot", name="ot")
                        nc.vector.tensor_add(ot[:], psm[m][:],
                                             acc[:, m:m + 1, :])
                        nc.sync.dma_start(out[m * 128:(m + 1) * 128, :],
                                          ot[:])

    nc.compile()
    return nc


_CACHE = {}


def _get_nc():
    if "nc" not in _CACHE:
        _CACHE["nc"] = _build()
    return _CACHE["nc"]


def _tile128(a):
    """[N, F] -> [128, N//128, F] with [p, nt, f] = a[nt*128 + p, f]."""
    n, f = a.shape
    return np.ascontiguousarray(
        a.reshape(n // 128, 128, f).transpose(1, 0, 2))


def make_in_maps(sae_features, emb, linear_w, linear_b, mlp1_w, mlp1_b,
                 mlp2_w, mlp2_b):
    bf = ml_dtypes.bfloat16
    f32 = np.float32
    sae = np.asarray(sae_features, dtype=f32)
    emb = np.asarray(emb, f32)

    saeT = np.ascontiguousarray(sae.T).astype(bf)          # (N, B)
    emb_bf = emb.astype(bf)                                 # (N, D)
    # FM stream operands are pre-scaled by powers of 2 (4*sae, 128*emb) so
    # their on-device squares land in fp8 e4m3 range for the DoubleRow
    # sum_sq GEMM; the 2^18 product scale is folded into the mlp1 weights
    # (inter_scaled = (512*se)^2 - 2^18*sq = 2^18 * 2*inter).
    sae_fm = (4.0 * sae).astype(bf)
    emb_fm = (128.0 * emb).astype(bf)
    mlp1wT = np.ascontiguousarray(
        ((0.5 / 2.0 ** 18) * np.asarray(mlp1_w, f32)).T)
    w1_t = _tile128(mlp1wT.astype(bf))                      # [128, DT, D]
    b1_t = np.asarray(mlp1_b, f32).reshape(1, D).astype(bf)     # [1, D]
    mlp2wT_f = np.ascontiguousarray(np.asarray(mlp2_w, f32).T)   # (D, O)
    linwT_f = np.ascontiguousarray(np.asarray(linear_w, f32).T)  # (N, O)
    bias_f = np.asarray(linear_b, f32) + np.asarray(mlp2_b, f32)  # (O,)

    in_maps = []
    for c in range(C):
        osl = slice(c * OS, (c + 1) * OS)
        bsl = slice(c * BS, (c + 1) * BS)
        fm = np.concatenate([np.ascontiguousarray(sae_fm.T)[:, bsl],
                             emb_fm], axis=1)              # (N, BS+D)
        lin = np.concatenate(
            [saeT, linwT_f[:, osl].astype(bf)], axis=1)       # (N, B+OS)
        in_maps.append({
            "fm_t": _tile128(fm),
            "lin_t": _tile128(lin),
            "w1_t": w1_t,
            "b1_t": b1_t,
            "w2_t": _tile128(
                np.ascontiguousarray(mlp2wT_f[:, osl]).astype(bf)),
            "biasrow": bias_f[osl].reshape(1, OS).astype(bf),
        })
    return in_maps


def kernel(sae_features, emb, linear_w, linear_b, mlp1_w, mlp1_b, mlp2_w,
           mlp2_b):
    nc = _get_nc()
    in_maps = make_in_maps(
        sae_features, emb, linear_w, linear_b, mlp1_w, mlp1_b, mlp2_w, mlp2_b
    )
    res = run_bass_kernel_spmd(nc, in_maps, list(range(C)))
    full = np.empty((B, O), dtype=np.float32)
    for c in range(C):
        full[:, c * OS:(c + 1) * OS] = res.results[c]["out"]
    return full


# revision 57
# speedup vs baseline: 1.1342x; 1.0249x over previous
"""NeuralFactorizationMachine Trainium2 kernel (8 NeuronCores, SPMD).

Reference computation (B=1024, N=16384, D=512, O=4096):
    sum_emb = sae @ emb                      (B, D)
    sum_sq  = (sae*sae) @ (emb*emb)          (B, D)
    inter   = 0.5*(sum_emb^2 - sum_sq)       (B, D)
    h       = relu(inter @ mlp1_w.T + b1)    (B, D)
    out     = h @ mlp2_w.T + b2 + sae @ linear_w.T + lb   (B, O)

Sharding (8 cores) — v2, collective-light:
  - FM is DATA-parallel over batch: each core computes sum_emb/sum_sq for
    its 128 batch rows over the FULL N using the full emb table. No
    AllReduce (the old N-sharded FM paid a ~225us 4MiB fp32 AllReduce).
  - linear / mlp2 are tensor-parallel over O (each core owns 512 output
    cols); final output is a host-side concat over O.
  - The only collective is a 1MiB bf16 AllGather of hT (the post-relu
    hidden layer), issued ~60us in and fully hidden under the linear GEMM.

Layouts: every heavy operand is PRE-TILED on the host into the exact
[128, ntiles, free] SBUF layout so all big DMAs are contiguous
(>=2KiB runs per partition, full 360GB/s, no descriptor penalty).

Phase order per core:
  1. FM: stream (saeb, emb) k-blocks; squares on DVE; 2x4 psum groups
     accumulate sum_embT/sum_sqT in [d, b] orientation (free dim = my 128
     batch cols) over 128 k-tiles.  interT = se^2 - sq (0.5 folded into
     mlp1 weights host-side), then mlp1 + relu+bias -> hT shard [D, 128].
  2. AllGather hT -> [8, D, 128] (every core now has h for all B).
  3. linear: stream (saeT, linwT) k-blocks, 8 psum groups accumulate
     out[m-tile, OS] over 128 k-tiles; then mlp2 (4 k-tiles of hT) and the
     rank-1 bias matmul accumulate into the SAME psum groups; one copy +
     store per m-tile.
All GEMM inputs are bf16 (fp32 PSUM accumulation).
"""

import numpy as np
import ml_dtypes

import concourse.bass as bass
import concourse.mybir as mybir
import concourse.tile as tile
from concourse import bacc
from concourse.bass_utils import run_bass_kernel_spmd

B, N, D, O = 1024, 16384, 512, 4096
C = 8                # cores
BS = B // C          # 128 batch rows per core (FM shard)
OS = O // C          # 512 output cols per core
BF16 = mybir.dt.bfloat16
F32 = mybir.dt.float32

NT = N // 128        # 128 k-tiles over the contraction dim
DT = D // 128        # 4 d-tiles
MT = B // 128        # 8 m-tiles (batch)
FMB = 16             # FM stream blocks
FILLK = 8            # linear blocks processed as FM-segment PE filler
F8 = mybir.dt.float8e4
FMT = NT // FMB      # 8 k-tiles per FM block
NB = 16              # linear stream blocks
NTB = NT // NB       # 8 k-tiles per linear block


def _build(phases=("fm", "ag", "lin", "mlp2")):
    nc = bacc.Bacc(
        "TRN2",
        target_bir_lowering=False,
        debug=False,
        enable_asserts=False,
        num_devices=C,
    )

    # fm_t packs [saeb | emb] along the free dim; lin_t packs [saeT | linwT].
    fm_t = nc.dram_tensor("fm_t", [128, NT, BS + D], BF16,
                          kind="ExternalInput").ap()
    lin_t = nc.dram_tensor("lin_t", [128, NT, B + OS], BF16,
                           kind="ExternalInput").ap()
    w1_t = nc.dram_tensor("w1_t", [128, DT, D], BF16, kind="ExternalInput").ap()
    b1_t = nc.dram_tensor("b1_t", [1, D], BF16, kind="ExternalInput").ap()
    w2_t = nc.dram_tensor("w2_t", [128, DT, OS], BF16, kind="ExternalInput").ap()
    biasrow = nc.dram_tensor("biasrow", [1, OS], BF16, kind="ExternalInput").ap()
    out = nc.dram_tensor("out", [B, OS], F32, kind="ExternalOutput").ap()

    with tile.TileContext(nc) as tc:
        with (
            tc.tile_pool(name="dram", bufs=1, space="DRAM") as dram,
            tc.tile_pool(name="wp", bufs=1) as wp,
            tc.tile_pool(name="p2s", bufs=3) as p2s,
        ):
            ag_in = dram.tile([D, BS], BF16, tag="ag_in", name="ag_in")
            ag_out = dram.tile([C, D, BS], BF16, tag="ag_out", name="ag_out",
                               addr_space="Shared")

            # small weights/constants on the scalar HWDGE queue so they never
            # queue behind the big sync-queue streams.
            w1 = wp.tile([128, DT, D], BF16, tag="w1", name="w1")
            with tc.tile_wait_until(0.09):
                nc.scalar.dma_start(w1[:], w1_t[:, :, :])
            b1 = wp.tile([1, D], BF16, tag="b1", name="b1")
            nc.scalar.dma_start(b1[:], b1_t[:, :])
            w2 = wp.tile([128, DT, OS], BF16, tag="w2", name="w2")
            with tc.tile_wait_until(0.15):
                nc.scalar.dma_start(w2[:], w2_t[:, :, :])
            br = wp.tile([1, OS], BF16, tag="br", name="br")
            nc.scalar.dma_start(br[:], biasrow[:, :])
            ones = wp.tile([1, 128], BF16, tag="ones", name="ones")
            nc.vector.memset(ones[:], 1.0)

            ht = wp.tile([128, DT, BS], BF16, tag="ht", name="ht")
            ht_all = wp.tile([128, DT, C, BS], BF16, tag="ht_all",
                             name="ht_all")

            # ---------------- Phase 1: FM (batch shard, full N) ------------
            # Linear block 0 is processed DURING the FM stream as PE filler:
            # m-sequential matmuls through 2 rotating PSUM banks, drained to
            # the SBUF accumulator acc.  FM's 8 accumulation groups are packed
            # into 2 PSUM banks so both coexist (4 of 8 banks used).
            lin_tiles = {}

            def load_lin_block(nb):
                ksl = slice(nb * NTB, (nb + 1) * NTB)
                t = p2s.tile([128, NTB, B + OS], BF16, tag="sl", name="sl")
                # Blocks >=1 are held back so the scheduler cannot hoist their
                # 9.5us transfers into the middle of the FM stream (which
                # would starve the FM PE wave); block 0 loads early as the
                # PE-filler operand.
                if nb == 0:
                    for q in range(4):   # finer deps: fills start sooner
                        nc.sync.dma_start(t[:, 2 * q:2 * q + 2, :],
                                          lin_t[:, 2 * q:2 * q + 2, :])
                elif nb < FILLK:
                    # quarters: fill matmuls chase each landing slice
                    for q in range(4):
                        nc.sync.dma_start(
                            t[:, 2 * q:2 * q + 2, :],
                            lin_t[:, nb * NTB + 2 * q:nb * NTB + 2 * q + 2, :])
                else:
                    with tc.tile_wait_until((125 + 9.0 * (nb - FILLK)) / 1000.0):
                        nc.sync.dma_start(t[:], lin_t[:, ksl, :])
                lin_tiles[nb] = t

            acc = wp.tile([128, MT, OS], F32, tag="acc", name="acc")

            # fill units: single blocks early (they become ready as each
            # block lands), pairs later (halves the drain traffic).
            FILL_UNITS = [(0, 1), (2, 3), (4, 5), (6, 7)]

            def filler(ju, m, fill_ps):
                blocks = FILL_UNITS[ju]
                ps = fill_ps.tile([128, OS], F32, tag="fill", name="fill")
                for h, nb in enumerate(blocks):
                    sl = lin_tiles[nb]
                    for nt in range(NTB):
                        nc.tensor.matmul(
                            ps[:],
                            sl[:, nt:nt + 1, m * 128:(m + 1) * 128],
                            sl[:, nt:nt + 1, B:B + OS],
                            start=(h == 0 and nt == 0),
                            stop=(h == len(blocks) - 1 and nt == NTB - 1),
                            skip_group_check=True,
                        )
                if ju == 0:
                    nc.scalar.copy(acc[:, m:m + 1, :], ps[:])
                else:
                    nc.vector.tensor_add(acc[:, m:m + 1, :],
                                         acc[:, m:m + 1, :], ps[:])

            if "fm" in phases:
                ident = wp.tile([128, 128], BF16, tag="ident", name="ident")
                from concourse.masks import make_identity
                make_identity(nc, ident[:])
                with (
                    tc.tile_pool(name="fms", bufs=3) as fms,
                    tc.tile_pool(name="fmps", bufs=2, space="PSUM") as fmps,
                    tc.tile_pool(name="fillps", bufs=2, space="PSUM") as fillps,
                    tc.tile_pool(name="fmst", bufs=4) as fmst,
                ):
                    # sum_emb / sum_sq in [my 128 batch rows, D] orientation:
                    # one full-bank psum accumulation group per GEMM.
                    pss = [fmps.tile([128, D], F32, tag="fmps",
                                     name=f"fmps{t}") for t in range(2)]
                    # small leading blocks so the PE wave starts sooner
                    sizes = [2, 2, 4] + [FMT] * (FMB - 1)
                    bounds = np.cumsum([0] + sizes)
                    nfill_units = len(FILL_UNITS) * MT \
                        if "lin" in phases else 0
                    fills = 0
                    next_lin = 0
                    nblk = len(sizes)
                    for blk, (k0, k1) in enumerate(zip(bounds[:-1],
                                                       bounds[1:])):
                        kn = k1 - k0
                        cb = fms.tile([128, FMT, BS + D], BF16, tag="cb",
                                      name="cb")
                        nc.sync.dma_start(cb[:, 0:kn, :], fm_t[:, k0:k1, :])
                        cq = fms.tile([128, FMT, BS + D], F8, tag="cq",
                                      name="cq")
                        # bf16 -> fp8 squares run at 1 elem/cycle (no DVE 2x
                        # mode with a 1-byte operand), so split across engines
                        nc.vector.tensor_mul(cq[:, 0:kn, 0:320],
                                             cb[:, 0:kn, 0:320],
                                             cb[:, 0:kn, 0:320])
                        nc.scalar.activation(
                            cq[:, 0:kn, 320:BS + D], cb[:, 0:kn, 320:BS + D],
                            mybir.ActivationFunctionType.Square)
                        if "lin" in phases and next_lin < FILLK:
                            load_lin_block(next_lin)
                            next_lin += 1
                        esl = slice(BS, BS + D)
                        for nt in range(kn):
                            nc.tensor.matmul(
                                pss[0][:],
                                cb[:, nt:nt + 1, 0:BS],
                                cb[:, nt:nt + 1, esl],
                                start=(k0 + nt == 0), stop=(k0 + nt == NT - 1),
                                skip_group_check=True,
                            )
                        # sum_sq in fp8 DoubleRow: 2 k-tiles per instruction
                        for nt in range(0, kn, 2):
                            nc.tensor.matmul(
                                pss[1][:],
                                cq[:, nt:nt + 2, 0:BS],
                                cq[:, nt:nt + 2, esl],
                                start=(k0 + nt == 0),
                                stop=(k0 + nt == NT - 2),
                                skip_group_check=True,
                                perf_mode=mybir.MatmulPerfMode.DoubleRow,
                            )
                        # spread the fill units across the stream (nb-major
                        # so each linear block's tile retires promptly)
                        ready_units = sum(
                            1 for u in FILL_UNITS if max(u) < next_lin)
                        target = (blk + 1) * nfill_units // nblk
                        while fills < min(target, ready_units * MT):
                            filler(fills // MT, fills % MT, fillps)
                            fills += 1
                    while fills < nfill_units:
                        filler(fills // MT, fills % MT, fillps)
                        fills += 1

                    # inter[b, d] = sum_emb^2 - sum_sq (0.5 folded into w1
                    # host-side), cast to bf16.  Square on the Act engine:
                    # a tensor op may read only ONE input from PSUM.
                    inter = wp.tile([128, D], BF16, tag="inter", name="inter")
                    tmp = fmst.tile([128, D], F32, tag="tmp", name="tmp")
                    nc.scalar.activation(tmp[:], pss[0][:],
                                         mybir.ActivationFunctionType.Square)
                    nc.vector.tensor_sub(inter[:], tmp[:], pss[1][:])

                # transpose inter -> interT tiles, then
                # hT[d2, b] = relu(mlp1wT.T @ interT + b1); bias applied via
                # rank-1 matmuls, d2 groups laid group-major in ONE psum bank
                # so a single relu activation covers all of hT.
                interT = wp.tile([128, DT, BS], BF16, tag="interT",
                                 name="interT")
                with tc.tile_pool(name="m1ps", bufs=2, space="PSUM") as m1ps:
                    for d in range(DT):
                        pt = m1ps.tile([128, BS], BF16, tag="trps",
                                       name="trps")
                        nc.tensor.transpose(
                            pt[:], inter[:, d * 128:(d + 1) * 128], ident[:])
                        nc.vector.tensor_copy(interT[:, d:d + 1, :], pt[:])
                    ps1 = m1ps.tile([128, DT * BS], F32, tag="m1ps",
                                    name="m1ps")
                    for d2 in range(DT):
                        osl = slice(d2 * BS, (d2 + 1) * BS)
                        for kd in range(DT):
                            nc.tensor.matmul(
                                ps1[:, osl],
                                w1[:, kd:kd + 1, d2 * 128:(d2 + 1) * 128],
                                interT[:, kd:kd + 1, :],
                                start=(kd == 0), stop=False,
                                skip_group_check=True,
                            )
                        nc.tensor.matmul(
                            ps1[:, osl], b1[0:1, d2 * 128:(d2 + 1) * 128], ones[:, :],
                            start=False, stop=True, skip_group_check=True,
                        )
                    nc.scalar.activation(
                        ht[:], ps1[:],
                        mybir.ActivationFunctionType.Relu,
                    )

                # ship my hT shard; AllGather; pull back the full hT.
                nc.gpsimd.dma_start(
                    ag_in.rearrange("(k p) b -> p k b", p=128), ht[:]
                )
                if "ag" in phases:
                    nc.gpsimd.collective_compute(
                        "AllGather",
                        mybir.AluOpType.bypass,
                        replica_groups=[list(range(C))],
                        ins=[ag_in.opt()],
                        outs=[ag_out.opt()],
                    )
                    for r in range(C):
                        nc.scalar.dma_start(
                            ht_all[:, :, r:r + 1, :],
                            ag_out[r:r + 1].rearrange(
                                "r (k p) b -> p (r k) b", p=128),
                        )
                else:
                    for r in range(C):
                        nc.scalar.dma_start(
                            ht_all[:, :, r:r + 1, :],
                            ag_in.rearrange("(k p) b -> p k b", p=128),
                        )

            # ---------------- Phase 2: linear GEMM + mlp2 (O shard) --------
            with (
                tc.tile_pool(name="p2ps", bufs=1, space="PSUM") as p2ps,
                tc.tile_pool(name="p2st", bufs=4) as p2st,
            ):
                psm = [p2ps.tile([128, OS], F32, tag=f"psm{m}", name=f"psm{m}")
                       for m in range(MT)]
                def mlp2_and_bias():
                    # accumulates into the open psm groups; hoisted mid-stream
                    # (order-independent) so it's off the critical-path tail.
                    for m in range(MT):
                        if "mlp2" in phases and "fm" in phases:
                            for kd in range(DT):
                                nc.tensor.matmul(
                                    psm[m][:],
                                    ht_all[:, kd:kd + 1, m:m + 1, :],
                                    w2[:, kd:kd + 1, :],
                                    start=False, stop=False,
                                    skip_group_check=True,
                                )
                        nc.tensor.matmul(
                            psm[m][:], ones[:, :], br[:, :],
                            start=False, stop=False, skip_group_check=True,
                        )

                if "lin" in phases:
                    nb0 = FILLK if "fm" in phases else 0
                    if "fm" in phases:
                        for j in range(FILLK):
                            lin_tiles.pop(j)  # consumed by the FM-phase filler
                    else:
                        nc.vector.memset(acc[:], 0.0)
                    for nb in range(nb0, NB):
                        if nb not in lin_tiles:
                            load_lin_block(nb)
                        sl = lin_tiles.pop(nb)
                        if nb < NB - 1:
                            for nt in range(NTB):
                                for m in range(MT):
                                    nc.tensor.matmul(
                                        psm[m][:],
                                        sl[:, nt:nt + 1,
                                           m * 128:(m + 1) * 128],
                                        sl[:, nt:nt + 1, B:B + OS],
                                        start=(nb == nb0 and nt == 0),
                                        stop=False,
                                        skip_group_check=True,
                                    )
                        else:
                            # last block m-major: each group's stop fires
                            # ~1.7us apart so the add+store pipeline drains
                            # under the remaining matmuls, not after them.
                            for m in range(MT):
                                for nt in range(NTB):
                                    nc.tensor.matmul(
                                        psm[m][:],
                                        sl[:, nt:nt + 1,
                                           m * 128:(m + 1) * 128],
                                        sl[:, nt:nt + 1, B:B + OS],
                                        start=False, stop=(nt == NTB - 1),
                                        skip_group_check=True,
                                    )
                                ot = p2st.tile([128, OS], F32, tag="ot",
                                               name="ot")
                                for hh in range(2):
                                    osl2 = slice(hh * OS // 2,
                                                 (hh + 1) * OS // 2)
                                    nc.vector.tensor_add(
                                        ot[:, osl2], psm[m][:, osl2],
                                        acc[:, m:m + 1, osl2])
                                    nc.sync.dma_start(
                                        out[m * 128:(m + 1) * 128, osl2],
                                        ot[:, osl2])
                        if nb == (nb0 + NB - 1) // 2:
                            mlp2_and_bias()
                else:
                    nc.vector.memset(acc[:], 0.0)
                    has_mlp2 = "mlp2" in phases and "fm" in phases
                    for m in range(MT):
                        nc.tensor.matmul(
                            psm[m][:], ones[:, :], br[:, :],
                            start=True, stop=not has_mlp2,
                            skip_group_check=True,
                        )
                        if has_mlp2:
                            for kd in range(DT):
                                nc.tensor.matmul(
                                    psm[m][:],
                                    ht_all[:, kd:kd + 1, m:m + 1, :],
                                    w2[:, kd:kd + 1, :],
                                    start=False, stop=(kd == DT - 1),
                                    skip_group_check=True,
                                )

                if "lin" not in phases:
                    for m in range(MT):
                        ot = p2st.tile([128, OS], F32, tag="ot", name="ot")
                        nc.vector.tensor_add(ot[:], psm[m][:],
                                             acc[:, m:m + 1, :])
                        nc.sync.dma_start(out[m * 128:(m + 1) * 128, :],
                                          ot[:])

    nc.compile()
    return nc


_CACHE = {}


def _get_nc():
    if "nc" not in _CACHE:
        _CACHE["nc"] = _build()
    return _CACHE["nc"]


def _tile128(a):
    """[N, F] -> [128, N//128, F] with [p, nt, f] = a[nt*128 + p, f]."""
    n, f = a.shape
    return np.ascontiguousarray(
        a.reshape(n // 128, 128, f).transpose(1, 0, 2))


def make_in_maps(sae_features, emb, linear_w, linear_b, mlp1_w, mlp1_b,
                 mlp2_w, mlp2_b):
    bf = ml_dtypes.bfloat16
    f32 = np.float32
    sae = np.asarray(sae_features, dtype=f32)
    emb = np.asarray(emb, f32)

    saeT = np.ascontiguousarray(sae.T).astype(bf)          # (N, B)
    emb_bf = emb.astype(bf)                                 # (N, D)
    # FM stream operands are pre-scaled by powers of 2 (4*sae, 128*emb) so
    # their on-device squares land in fp8 e4m3 range for the DoubleRow
    # sum_sq GEMM; the 2^18 product scale is folded into the mlp1 weights
    # (inter_scaled = (512*se)^2 - 2^18*sq = 2^18 * 2*inter).
    sae_fm = (4.0 * sae).astype(bf)
    emb_fm = (128.0 * emb).astype(bf)
    mlp1wT = np.ascontiguousarray(
        ((0.5 / 2.0 ** 18) * np.asarray(mlp1_w, f32)).T)
    w1_t = _tile128(mlp1wT.astype(bf))                      # [128, DT, D]
    b1_t = np.asarray(mlp1_b, f32).reshape(1, D).astype(bf)     # [1, D]
    mlp2wT_f = np.ascontiguousarray(np.asarray(mlp2_w, f32).T)   # (D, O)
    linwT_f = np.ascontiguousarray(np.asarray(linear_w, f32).T)  # (N, O)
    bias_f = np.asarray(linear_b, f32) + np.asarray(mlp2_b, f32)  # (O,)

    in_maps = []
    for c in range(C):
        osl = slice(c * OS, (c + 1) * OS)
        bsl = slice(c * BS, (c + 1) * BS)
        fm = np.concatenate([np.ascontiguousarray(sae_fm.T)[:, bsl],
                             emb_fm], axis=1)              # (N, BS+D)
        lin = np.concatenate(
            [saeT, linwT_f[:, osl].astype(bf)], axis=1)       # (N, B+OS)
        in_maps.append({
            "fm_t": _tile128(fm),
            "lin_t": _tile128(lin),
            "w1_t": w1_t,
            "b1_t": b1_t,
            "w2_t": _tile128(
                np.ascontiguousarray(mlp2wT_f[:, osl]).astype(bf)),
            "biasrow": bias_f[osl].reshape(1, OS).astype(bf),
        })
    return in_maps


def kernel(sae_features, emb, linear_w, linear_b, mlp1_w, mlp1_b, mlp2_w,
           mlp2_b):
    nc = _get_nc()
    in_maps = make_in_maps(
        sae_features, emb, linear_w, linear_b, mlp1_w, mlp1_b, mlp2_w, mlp2_b
    )
    res = run_bass_kernel_spmd(nc, in_maps, list(range(C)))
    full = np.empty((B, O), dtype=np.float32)
    for c in range(C):
        full[:, c * OS:(c + 1) * OS] = res.results[c]["out"]
    return full
